# revision 18
# baseline (speedup 1.0000x reference)
"""TRN2 Bass kernel for nn_CIN (2-layer Compressed Interaction Network), v7.

Reference computation (per sample b):
  inter0[(p,q),d] = xe[b,p,d] * xe[b,q,d]          (F=39 fields, D=16)
  x1[h,d]  = sum_{p,q} W0[h, p*39+q] * inter0[(p,q),d]   (h=128)
  out0[h]  = sum_d x1[h,d]
  out1[h]  = sum_{i,j} W1[h,i,j] * G[i,j],  G[i,j] = sum_d x1[i,d]*xe[b,j,d]
  out = concat(out0, out1)    -> [B, 256]

v7 strategy (8-core data parallel, 256 samples/core):
  * Host ships the symmetrized interaction tensor INTER[pair,(b,d)] in
    fp8-e3m4 with per-column (b,d) exact scales; inverse scales fold into
    the xe diagonal blocks host-side (exact cancellation in G / out0).
  * Layer-0: per 8-sample block, 7 chunk matmuls (stationary = fp8 INTER
    block, moving = f16 W0sym chunk) accumulate x1 as [(s,d), h].
  * G operand (block-diag-8 xe + ones) is assembled on device: engine
    memsets write the zeros (DVE + Pool halves), one DMA per s'-row-block
    writes the dense diagonal blocks -- only 0.65MB of xe bytes move.
  * G matmuls run two pieces behind layer-0 so the xcd assembly and W1
    transfers never stall the in-order PE queue.
  * out1 via two half-batch chains of 39 field matmuls; ones column of G
    yields out0. Outputs land packed [128, half, kind, 128]; host
    transposes/assembles.
  * DMA issue times are pinned with tile_wait_until to serialize the
    shared DMA device in bandwidth-priority order.
"""

import os
import sys

sys.path.insert(0, "/opt/trn_rl_repo")

import numpy as np

F16 = np.float16

NUM_FIELD = 39
H = 128            # CIN layer width (both layers)
D = 16             # embed dim
BATCH = 2048
NCORES = 8
B_LOC = BATCH // NCORES          # 256
BD = B_LOC * D                   # 4096 columns, b-major / d-minor
NPAIR = 780                      # unique (p<=q) pairs
CS = 112                         # chunk rows
NCH = 7                          # 7 * 112 = 784 (4 zero pad rows)
NPAD = CS * NCH                  # 784
NGRP = B_LOC // 8                # 32 groups of 8 samples
GW = 40                          # 39 fields + ones column
# pieces of sample-groups streamed through SBUF (sum = 32 groups)
PIECES = [2, 2, 4, 4, 4, 4, 4, 4, 2, 2]
GLAG = 3                         # max G lag behind layer-0 (early pieces)
# G pieces emitted after each layer-0 piece (index = pc); rest in the tail
G_AT = [[], [], [], [0], [1], [2, 3], [4, 5], [6], [7, 8], []]
G_TAIL = [9]
# pinned DMA issue times (ns; transfer ~= pin + 1.3us, serialized on device)
T_PIECE = [0, 850, 3200, 5700, 8400, 11800, 14200, 17100, 18200, 18750]
T_XCD4 = [1400, 4300, 10000, 15750]
T_W1A = 7100
T_W1B = 13000
N_WARM = int(os.environ.get("KWARM", "10"))


def _pairs():
    ps, qs = [], []
    for p in range(NUM_FIELD):
        for q in range(p, NUM_FIELD):
            ps.append(p)
            qs.append(q)
    return np.array(ps), np.array(qs)


_P_IDX, _Q_IDX = _pairs()

_COMPILED = None


def _build_module():
    import concourse.bass as bass
    import concourse.bacc as bacc
    import concourse.mybir as mybir
    from concourse import tile

    f32 = mybir.dt.float32
    f16 = mybir.dt.float16
    f8e3 = mybir.dt.float8e3

    nc = bacc.Bacc("TRN2", target_bir_lowering=False, debug=False)

    INTER = nc.dram_tensor("INTER", [CS, NCH, BD], f8e3, kind="ExternalInput")
    W0P = nc.dram_tensor("W0P", [CS, NCH, H], f16, kind="ExternalInput")
    XCD = nc.dram_tensor("XCD", [128, NGRP, 8, GW], f16, kind="ExternalInput")
    W1T = nc.dram_tensor("W1T", [128, NUM_FIELD, H], f16, kind="ExternalInput")
    # OUT[p, half, 0, :] = out0 cols (i=p), OUT[p, half, 1, :] = out1 rows (b=p)
    OUT = nc.dram_tensor("OUT", [128, 2, 2, H], f16, kind="ExternalOutput")
    US = 1e-6  # tile_wait_until takes ms; T_* values are in ns

    with tile.TileContext(nc) as tc:
        with tc.tile_pool(name="const", bufs=1) as cpool, \
             tc.tile_pool(name="inter", bufs=2) as ipool, \
             tc.tile_pool(name="psX", bufs=2, space="PSUM") as psX, \
             tc.tile_pool(name="psG", bufs=2, space="PSUM") as psG, \
             tc.tile_pool(name="psO", bufs=2, space="PSUM") as psO:

            w0p = cpool.tile([CS, NCH, H], f16, tag="w0p")

            xcd = cpool.tile([128, NGRP, 8, GW], f16, tag="xcd")
            w1t = cpool.tile([128, NUM_FIELD, H], f16, tag="w1t")

            # persistent intermediates
            x1tall = cpool.tile([128, NGRP, H], f16, tag="x1tall")
            gsb = cpool.tile([128, NGRP, 8, GW], f16, tag="gsb")
            obuf = cpool.tile([128, 2, 2, 128], f16, tag="obuf")

            # ---- xcd: full block-diag ships in 4 group-quarter DMAs ----
            engs = [nc.gpsimd, nc.scalar, nc.gpsimd, nc.scalar]
            for k in range(4):
                with tc.tile_wait_until(T_XCD4[k] * US):
                    engs[k].dma_start(xcd[:, 8 * k:8 * (k + 1)],
                                      XCD[:, 8 * k:8 * (k + 1)])

            # ---- PE warm-up: matmuls on a memset scratch tile ----
            wtile = cpool.tile([128, 256], f16, tag="wtile")
            nc.vector.memset(wtile[:], 0.25)
            warm = psX.tile([128, 4, H], f32, tag="x1p")
            wv = warm[:].rearrange("p a n -> p (a n)")
            for _ in range(N_WARM):
                nc.tensor.matmul(wv[:, 0:256], wtile[:, 0:128], wtile[:],
                                 start=True, stop=True)

            def emit_g(pc, npg, g0):
                """G matmuls + gsb drains for piece pc (npg groups at g0)."""
                for gg_ in range(npg // 2):
                    gg = gg_ + pc
                    gps = psG.tile([128, 2, 512], f32, tag="gps")
                    for g2 in range(2):
                        gi = g0 + gg_ * 2 + g2
                        nc.tensor.matmul(
                            gps[:, g2, 0:8 * GW],
                            x1tall[:, gi, :],
                            xcd[:, gi, :, :],
                            start=True, stop=True,
                        )
                    gi = g0 + gg_ * 2
                    gview = gps[:, :, 0:8 * GW].rearrange(
                        "p a (s j) -> p a s j", j=GW)
                    if gg % 2 == 0:
                        nc.vector.tensor_copy(gsb[:, gi:gi + 2, :, :], gview)
                    else:
                        nc.scalar.copy(gsb[:, gi:gi + 2, :, :], gview)

            def emit_chain(h):
                """out1 chain + out0 extraction for sample half h."""
                o1 = psO.tile([128, H], f32, tag="o1ps")
                gslice = gsb[:, 16 * h:16 * (h + 1), :, :]
                for j in range(NUM_FIELD):
                    nc.tensor.matmul(
                        o1[:],
                        gslice[:, :, :, j],          # [128 i, (16 g, 8 s)]
                        w1t[:, j, :],                # [128 i, 128 h]
                        start=(j == 0), stop=(j == NUM_FIELD - 1),
                    )
                nc.scalar.copy(obuf[:, h, 1, :], o1[:])
                # out0: ones-column gather [128 i, 16 g, 8 s]
                nc.vector.tensor_copy(obuf[:, h, 0, :],
                                      gslice[:, :, :, GW - 1])
                (nc.gpsimd if h == 0 else nc.sync).dma_start(
                    OUT[:, h], obuf[:, h])

            # ---- main pipeline: layer-0 streams; G trails by GLAG pieces ----
            goff = [0]
            for i in range(len(PIECES) - 1):
                goff.append(goff[-1] + PIECES[i])

            g0 = 0
            for pc, npg in enumerate(PIECES):
                cols = npg * 8 * D                  # 512 or 256
                c0 = g0 * 8 * D
                itag = f"ip{npg}"
                intp = ipool.tile([CS, NCH, cols], f8e3, tag=itag)
                with tc.tile_wait_until(T_PIECE[pc] * US):
                    if pc % 2 == 0:
                        nc.sync.dma_start(intp[:], INTER[:, :, c0:c0 + cols])
                    else:
                        nc.gpsimd.dma_start(intp[:], INTER[:, :, c0:c0 + cols])
                if pc == 0:
                    with tc.tile_wait_until(500 * US):
                        nc.sync.dma_start(w0p[:], W0P[:])
                if pc == 1:
                    with tc.tile_wait_until(T_W1A * US):
                        nc.sync.dma_start(w1t[:, 0:20, :], W1T[:, 0:20, :])
                if pc == 2:
                    with tc.tile_wait_until(T_W1B * US):
                        nc.sync.dma_start(w1t[:, 20:NUM_FIELD, :],
                                          W1T[:, 20:NUM_FIELD, :])

                x1f = psX.tile([128, 4, H], f32, tag="x1p")
                x1 = x1f[:, 0:npg, :]
                for gl in range(npg):
                    for c in range(NCH):
                        nc.tensor.matmul(
                            x1[:, gl, :],
                            intp[:, c, 128 * gl:128 * (gl + 1)],
                            w0p[:, c, :],
                            start=(c == 0), stop=(c == NCH - 1),
                        )
                if pc % 2 == 0:
                    nc.scalar.copy(x1tall[:, g0:g0 + npg, :], x1[:])
                else:
                    nc.vector.tensor_copy(x1tall[:, g0:g0 + npg, :], x1[:])

                for gpc in G_AT[pc]:
                    emit_g(gpc, PIECES[gpc], goff[gpc])
                    if goff[gpc] + PIECES[gpc] == 16:
                        emit_chain(0)
                g0 += npg

            for gpc in G_TAIL:
                emit_g(gpc, PIECES[gpc], goff[gpc])
            emit_chain(1)

    nc.compile()
    return nc


def _host_prep(x_emb, W0, W1):
    """Build per-core input maps."""
    import ml_dtypes
    E3 = ml_dtypes.float8_e3m4

    # symmetrized, chunk-packed W0
    W0m = W0.reshape(H, NUM_FIELD, NUM_FIELD)
    W0sym = W0m[:, _P_IDX, _Q_IDX] + np.where(
        (_P_IDX != _Q_IDX)[None, :], W0m[:, _Q_IDX, _P_IDX], 0.0
    )                                            # [H, 780]
    W0p = np.zeros((NPAD, H), np.float32)
    W0p[:NPAIR] = W0sym.T
    w0p = np.ascontiguousarray(
        W0p.reshape(NCH, CS, H).transpose(1, 0, 2)).astype(F16)

    w1t = np.ascontiguousarray(
        W1.reshape(H, H, NUM_FIELD).transpose(1, 2, 0)).astype(F16)  # [i,j,h]

    maps = []
    for core in range(NCORES):
        xe = x_emb[core * B_LOC:(core + 1) * B_LOC]          # [256, 39, 16]
        xT = np.ascontiguousarray(xe.transpose(1, 0, 2)).reshape(NUM_FIELD, BD)

        prod = xT[_P_IDX] * xT[_Q_IDX]                       # [780, 4096] f32
        # per-column (b,d) scales, f16-rounded; fold inverse into xcdiag
        colmax = np.abs(prod).max(axis=0)
        svec = np.maximum(colmax / 15.5, 2.0 ** -14).astype(F16)  # [4096]
        prodq = np.clip(prod / svec.astype(np.float32)[None, :],
                        -15.5, 15.5)
        prodp = np.zeros((NPAD, BD), np.float32)
        prodp[:NPAIR] = prodq
        inter = np.ascontiguousarray(
            prodp.reshape(NCH, CS, BD).transpose(1, 0, 2)).astype(E3)

        # block-diag-8 (xe*svec | svec ones): [(s,d), g, 8, 40]
        sv = svec.astype(np.float32).reshape(B_LOC, D)       # [b, d]
        xcd = np.zeros((128, NGRP, 8, GW), np.float32)
        xe_t = xe.transpose(0, 2, 1)                         # [b, d, j]
        for s in range(8):
            rows = slice(s * D, (s + 1) * D)
            svs = sv[s::8].T[:, :, None]                     # [d, g, 1]
            xcd[rows, :, s, 0:NUM_FIELD] = xe_t[s::8].transpose(1, 0, 2) * svs
            xcd[rows, :, s, GW - 1] = svs[:, :, 0]
        xcd = xcd.astype(F16)

        maps.append({
            "INTER": inter, "W0P": w0p, "XCD": xcd, "W1T": w1t,
        })
    return maps


def kernel(x_emb, W0, W1, _trace=False, _trace_kwargs=None):
    global _COMPILED
    if _COMPILED is None:
        _COMPILED = _build_module()
    nc = _COMPILED

    from concourse.bass_utils import run_bass_kernel_spmd

    in_maps = _host_prep(np.asarray(x_emb, np.float32),
                         np.asarray(W0, np.float32),
                         np.asarray(W1, np.float32))
    kw = {}
    if _trace:
        kw["trace"] = True
        kw.update(_trace_kwargs or {})
    res = run_bass_kernel_spmd(nc, in_maps, list(range(NCORES)), **kw)
    outs = []
    for i in range(NCORES):
        o = res.results[i]["OUT"].astype(np.float32)         # [128, 2, 2, 128]
        o0 = np.concatenate([o[:, 0, 0, :].T, o[:, 1, 0, :].T], axis=0)
        o1 = np.concatenate([o[:, 0, 1, :], o[:, 1, 1, :]], axis=0)
        outs.append(np.concatenate([o0, o1], axis=1))
    outp = np.concatenate(outs, axis=0)
    if _trace:
        return outp, res
    return outp


# revision 19
# speedup vs baseline: 1.0284x; 1.0284x over previous
"""TRN2 Bass kernel for nn_CIN (2-layer Compressed Interaction Network), v7.

Reference computation (per sample b):
  inter0[(p,q),d] = xe[b,p,d] * xe[b,q,d]          (F=39 fields, D=16)
  x1[h,d]  = sum_{p,q} W0[h, p*39+q] * inter0[(p,q),d]   (h=128)
  out0[h]  = sum_d x1[h,d]
  out1[h]  = sum_{i,j} W1[h,i,j] * G[i,j],  G[i,j] = sum_d x1[i,d]*xe[b,j,d]
  out = concat(out0, out1)    -> [B, 256]

v7 strategy (8-core data parallel, 256 samples/core):
  * Host ships the symmetrized interaction tensor INTER[pair,(b,d)] in
    fp8-e3m4 with per-column (b,d) exact scales; inverse scales fold into
    the xe diagonal blocks host-side (exact cancellation in G / out0).
  * Layer-0: per 8-sample block, 7 chunk matmuls (stationary = fp8 INTER
    block, moving = f16 W0sym chunk) accumulate x1 as [(s,d), h].
  * G operand (block-diag-8 xe + ones) is assembled on device: engine
    memsets write the zeros (DVE + Pool halves), one DMA per s'-row-block
    writes the dense diagonal blocks -- only 0.65MB of xe bytes move.
  * G matmuls run two pieces behind layer-0 so the xcd assembly and W1
    transfers never stall the in-order PE queue.
  * out1 via two half-batch chains of 39 field matmuls; ones column of G
    yields out0. Outputs land packed [128, half, kind, 128]; host
    transposes/assembles.
  * DMA issue times are pinned with tile_wait_until to serialize the
    shared DMA device in bandwidth-priority order.
"""

import os
import sys

sys.path.insert(0, "/opt/trn_rl_repo")

import numpy as np

F16 = np.float16

NUM_FIELD = 39
H = 128            # CIN layer width (both layers)
D = 16             # embed dim
BATCH = 2048
NCORES = 8
B_LOC = BATCH // NCORES          # 256
BD = B_LOC * D                   # 4096 columns, b-major / d-minor
NPAIR = 780                      # unique (p<=q) pairs
CS = 112                         # chunk rows
NCH = 7                          # 7 * 112 = 784 (4 zero pad rows)
NPAD = CS * NCH                  # 784
NGRP = B_LOC // 8                # 32 groups of 8 samples
GW = 40                          # 39 fields + ones column
# pieces of sample-groups streamed through SBUF (sum = 32 groups)
PIECES = [2, 2, 4, 4, 4, 4, 4, 4, 2, 2]
GLAG = 3                         # max G lag behind layer-0 (early pieces)
# G pieces emitted after each layer-0 piece (index = pc); rest in the tail
G_AT = [[], [], [], [0], [1], [2, 3], [4, 5], [6], [7, 8], []]
G_TAIL = [9]
# pinned DMA issue times (ns; transfer ~= pin + 1.3us, serialized on device)
T_PIECE = [0, 850, 3200, 5700, 8400, 13000, 14200, 17100, 18200, 18750]
T_XCD4 = [1400, 4300, 10000, 15750]
T_W1A = 7100
T_W1B = 11800
N_WARM = int(os.environ.get("KWARM", "10"))


def _pairs():
    ps, qs = [], []
    for p in range(NUM_FIELD):
        for q in range(p, NUM_FIELD):
            ps.append(p)
            qs.append(q)
    return np.array(ps), np.array(qs)


_P_IDX, _Q_IDX = _pairs()

_COMPILED = None


def _build_module():
    import concourse.bass as bass
    import concourse.bacc as bacc
    import concourse.mybir as mybir
    from concourse import tile

    f32 = mybir.dt.float32
    f16 = mybir.dt.float16
    f8e3 = mybir.dt.float8e3

    nc = bacc.Bacc("TRN2", target_bir_lowering=False, debug=False)

    INTER = nc.dram_tensor("INTER", [CS, NCH, BD], f8e3, kind="ExternalInput")
    W0P = nc.dram_tensor("W0P", [CS, NCH, H], f16, kind="ExternalInput")
    XCD = nc.dram_tensor("XCD", [128, NGRP, 8, GW], f16, kind="ExternalInput")
    W1T = nc.dram_tensor("W1T", [128, NUM_FIELD, H], f16, kind="ExternalInput")
    # OUT[p, half, 0, :] = out0 cols (i=p), OUT[p, half, 1, :] = out1 rows (b=p)
    OUT = nc.dram_tensor("OUT", [128, 2, 2, H], f16, kind="ExternalOutput")
    US = 1e-6  # tile_wait_until takes ms; T_* values are in ns

    with tile.TileContext(nc) as tc:
        with tc.tile_pool(name="const", bufs=1) as cpool, \
             tc.tile_pool(name="inter", bufs=2) as ipool, \
             tc.tile_pool(name="psX", bufs=2, space="PSUM") as psX, \
             tc.tile_pool(name="psG", bufs=2, space="PSUM") as psG, \
             tc.tile_pool(name="psO", bufs=2, space="PSUM") as psO:

            w0p = cpool.tile([CS, NCH, H], f16, tag="w0p")

            xcd = cpool.tile([128, NGRP, 8, GW], f16, tag="xcd")
            w1t = cpool.tile([128, NUM_FIELD, H], f16, tag="w1t")

            # persistent intermediates
            x1tall = cpool.tile([128, NGRP, H], f16, tag="x1tall")
            gsb = cpool.tile([128, NGRP, 8, GW], f16, tag="gsb")
            obuf = cpool.tile([128, 2, 2, 128], f16, tag="obuf")

            # ---- xcd: full block-diag ships in 4 group-quarter DMAs ----
            engs = [nc.gpsimd, nc.scalar, nc.gpsimd, nc.scalar]
            for k in range(4):
                with tc.tile_wait_until(T_XCD4[k] * US):
                    engs[k].dma_start(xcd[:, 8 * k:8 * (k + 1)],
                                      XCD[:, 8 * k:8 * (k + 1)])

            # ---- PE warm-up: matmuls on a memset scratch tile ----
            wtile = cpool.tile([128, 256], f16, tag="wtile")
            nc.vector.memset(wtile[:], 0.25)
            warm = psX.tile([128, 4, H], f32, tag="x1p")
            wv = warm[:].rearrange("p a n -> p (a n)")
            for _ in range(N_WARM):
                nc.tensor.matmul(wv[:, 0:256], wtile[:, 0:128], wtile[:],
                                 start=True, stop=True)

            def emit_g(pc, npg, g0):
                """G matmuls + gsb drains for piece pc (npg groups at g0)."""
                for gg_ in range(npg // 2):
                    gg = gg_ + pc
                    gps = psG.tile([128, 2, 512], f32, tag="gps")
                    for g2 in range(2):
                        gi = g0 + gg_ * 2 + g2
                        nc.tensor.matmul(
                            gps[:, g2, 0:8 * GW],
                            x1tall[:, gi, :],
                            xcd[:, gi, :, :],
                            start=True, stop=True,
                        )
                    gi = g0 + gg_ * 2
                    gview = gps[:, :, 0:8 * GW].rearrange(
                        "p a (s j) -> p a s j", j=GW)
                    if gg % 2 == 0:
                        nc.vector.tensor_copy(gsb[:, gi:gi + 2, :, :], gview)
                    else:
                        nc.scalar.copy(gsb[:, gi:gi + 2, :, :], gview)

            def emit_chain(h):
                """out1 chain + out0 extraction for sample half h."""
                o1 = psO.tile([128, H], f32, tag="o1ps")
                gslice = gsb[:, 16 * h:16 * (h + 1), :, :]
                for j in range(NUM_FIELD):
                    nc.tensor.matmul(
                        o1[:],
                        gslice[:, :, :, j],          # [128 i, (16 g, 8 s)]
                        w1t[:, j, :],                # [128 i, 128 h]
                        start=(j == 0), stop=(j == NUM_FIELD - 1),
                    )
                nc.scalar.copy(obuf[:, h, 1, :], o1[:])
                # out0: ones-column gather [128 i, 16 g, 8 s]
                nc.vector.tensor_copy(obuf[:, h, 0, :],
                                      gslice[:, :, :, GW - 1])
                (nc.gpsimd if h == 0 else nc.sync).dma_start(
                    OUT[:, h], obuf[:, h])

            # ---- main pipeline: layer-0 streams; G trails by GLAG pieces ----
            goff = [0]
            for i in range(len(PIECES) - 1):
                goff.append(goff[-1] + PIECES[i])

            g0 = 0
            for pc, npg in enumerate(PIECES):
                cols = npg * 8 * D                  # 512 or 256
                c0 = g0 * 8 * D
                itag = f"ip{npg}"
                intp = ipool.tile([CS, NCH, cols], f8e3, tag=itag)
                with tc.tile_wait_until(T_PIECE[pc] * US):
                    if pc % 2 == 0:
                        nc.sync.dma_start(intp[:], INTER[:, :, c0:c0 + cols])
                    else:
                        nc.gpsimd.dma_start(intp[:], INTER[:, :, c0:c0 + cols])
                if pc == 0:
                    with tc.tile_wait_until(500 * US):
                        nc.sync.dma_start(w0p[:], W0P[:])
                if pc == 1:
                    with tc.tile_wait_until(T_W1A * US):
                        nc.sync.dma_start(w1t[:, 0:20, :], W1T[:, 0:20, :])
                if pc == 2:
                    with tc.tile_wait_until(T_W1B * US):
                        nc.sync.dma_start(w1t[:, 20:NUM_FIELD, :],
                                          W1T[:, 20:NUM_FIELD, :])

                x1f = psX.tile([128, 4, H], f32, tag="x1p")
                x1 = x1f[:, 0:npg, :]
                for gl in range(npg):
                    for c in range(NCH):
                        nc.tensor.matmul(
                            x1[:, gl, :],
                            intp[:, c, 128 * gl:128 * (gl + 1)],
                            w0p[:, c, :],
                            start=(c == 0), stop=(c == NCH - 1),
                        )
                if pc % 2 == 0:
                    nc.scalar.copy(x1tall[:, g0:g0 + npg, :], x1[:])
                else:
                    nc.vector.tensor_copy(x1tall[:, g0:g0 + npg, :], x1[:])

                for gpc in G_AT[pc]:
                    emit_g(gpc, PIECES[gpc], goff[gpc])
                    if goff[gpc] + PIECES[gpc] == 16:
                        emit_chain(0)
                g0 += npg

            for gpc in G_TAIL:
                emit_g(gpc, PIECES[gpc], goff[gpc])
            emit_chain(1)

    nc.compile()
    return nc


def _host_prep(x_emb, W0, W1):
    """Build per-core input maps."""
    import ml_dtypes
    E3 = ml_dtypes.float8_e3m4

    # symmetrized, chunk-packed W0
    W0m = W0.reshape(H, NUM_FIELD, NUM_FIELD)
    W0sym = W0m[:, _P_IDX, _Q_IDX] + np.where(
        (_P_IDX != _Q_IDX)[None, :], W0m[:, _Q_IDX, _P_IDX], 0.0
    )                                            # [H, 780]
    W0p = np.zeros((NPAD, H), np.float32)
    W0p[:NPAIR] = W0sym.T
    w0p = np.ascontiguousarray(
        W0p.reshape(NCH, CS, H).transpose(1, 0, 2)).astype(F16)

    w1t = np.ascontiguousarray(
        W1.reshape(H, H, NUM_FIELD).transpose(1, 2, 0)).astype(F16)  # [i,j,h]

    maps = []
    for core in range(NCORES):
        xe = x_emb[core * B_LOC:(core + 1) * B_LOC]          # [256, 39, 16]
        xT = np.ascontiguousarray(xe.transpose(1, 0, 2)).reshape(NUM_FIELD, BD)

        prod = xT[_P_IDX] * xT[_Q_IDX]                       # [780, 4096] f32
        # per-column (b,d) scales, f16-rounded; fold inverse into xcdiag
        colmax = np.abs(prod).max(axis=0)
        svec = np.maximum(colmax / 15.5, 2.0 ** -14).astype(F16)  # [4096]
        prodq = np.clip(prod / svec.astype(np.float32)[None, :],
                        -15.5, 15.5)
        prodp = np.zeros((NPAD, BD), np.float32)
        prodp[:NPAIR] = prodq
        inter = np.ascontiguousarray(
            prodp.reshape(NCH, CS, BD).transpose(1, 0, 2)).astype(E3)

        # block-diag-8 (xe*svec | svec ones): [(s,d), g, 8, 40]
        sv = svec.astype(np.float32).reshape(B_LOC, D)       # [b, d]
        xcd = np.zeros((128, NGRP, 8, GW), np.float32)
        xe_t = xe.transpose(0, 2, 1)                         # [b, d, j]
        for s in range(8):
            rows = slice(s * D, (s + 1) * D)
            svs = sv[s::8].T[:, :, None]                     # [d, g, 1]
            xcd[rows, :, s, 0:NUM_FIELD] = xe_t[s::8].transpose(1, 0, 2) * svs
            xcd[rows, :, s, GW - 1] = svs[:, :, 0]
        xcd = xcd.astype(F16)

        maps.append({
            "INTER": inter, "W0P": w0p, "XCD": xcd, "W1T": w1t,
        })
    return maps


def kernel(x_emb, W0, W1, _trace=False, _trace_kwargs=None):
    global _COMPILED
    if _COMPILED is None:
        _COMPILED = _build_module()
    nc = _COMPILED

    from concourse.bass_utils import run_bass_kernel_spmd

    in_maps = _host_prep(np.asarray(x_emb, np.float32),
                         np.asarray(W0, np.float32),
                         np.asarray(W1, np.float32))
    kw = {}
    if _trace:
        kw["trace"] = True
        kw.update(_trace_kwargs or {})
    res = run_bass_kernel_spmd(nc, in_maps, list(range(NCORES)), **kw)
    outs = []
    for i in range(NCORES):
        o = res.results[i]["OUT"].astype(np.float32)         # [128, 2, 2, 128]
        o0 = np.concatenate([o[:, 0, 0, :].T, o[:, 1, 0, :].T], axis=0)
        o1 = np.concatenate([o[:, 0, 1, :], o[:, 1, 1, :]], axis=0)
        outs.append(np.concatenate([o0, o1], axis=1))
    outp = np.concatenate(outs, axis=0)
    if _trace:
        return outp, res
    return outp
